# revision 49
# baseline (speedup 1.0000x reference)
"""Trainium2 Bass kernel for nn_ALMSLayer (gnn_message_passing), 8 NeuronCores.

Algorithm (per core c, rows R_c = [c*1024, (c+1)*1024) of B=8192):
  x       = f / ||f||                      (rows normalized)
  sim     = x_c @ x^T                      (bf16 matmul, [1024, 8192])
  topk    : per row, 33rd/34th-largest via chunked top-8 candidates ->
            threshold t; M' = (sim >= t) in {0,1} (includes self edge)
  A       = (M' + M'^T)/32 - I/16          (-I/16 corrects the self edges)
  diff1   = A @ f ; geodesic = A @ diff1   (dense bf16 matmuls; M'^T side
            summed across cores with ReduceScatter, M' side local, full
            tensors rebuilt with a bf16 AllGather)
  z       = x + 0.1 * geodesic/||geodesic||
  out     = softmax((x_c @ z^T)/0.1) @ f   (flash-attention style)

Key perf structure vs the naive layout:
  * Masks never touch DRAM.  M' (natural, fp8) lives in SBUF; M'^T j-slices
    are recomputed on the fly from sim^T = xbT_js^T @ xcT (one PE matmul)
    with the per-i threshold applied by a rank-1 PSUM accumulate + sigmoid
    (even slices) or a broadcast-threshold compare on DVE (odd); slices
    [0, NRES) are kept resident, the tail is recomputed again in gather-2
    where it hides under the concurrent ReduceScatter.
  * The scatter bounce is kept TRANSPOSED [8*128 d, BC j] so ReduceScatter
    block c needs no pre-transpose; rs^T merges into G^T before a single
    8-transpose pass back to natural layout.
  * All [N*128, D] feature DMAs use a per-1024-block p-major DRAM layout
    (host pre-permutes; labels cancel so the output needs no un-permute),
    giving 4 KB contiguous descriptors instead of 256-512 B.
  * x^T and z^T full tensors are built by AllGathering locally PE-transposed
    [D, 1024] chunks (natural-layout loads), never by DMA transpose.
  * diff2's scatter is issued before diff1's AllGather completes; the sync
    DMA queue is kept clear of bounce writes so diffusion-2's den load
    fires the moment the AllGather lands.
  * Phase 8 warms the HAM clock gate with real matmuls (transpose-mode
    does not count as PE activity) so scoring starts at 2.4 GHz.

Host side only shards/permutes/replicates/concats; all arithmetic on device.
"""
import contextlib
import sys

sys.path.insert(0, "/opt/trn_rl_repo")

import numpy as np

import concourse.bass as bass
import concourse.tile as tile
from concourse import bacc, mybir
from concourse.bass_utils import run_bass_kernel_spmd

F32 = mybir.dt.float32
BF16 = mybir.dt.bfloat16
FP8 = mybir.dt.float8e4
AF = mybir.ActivationFunctionType
ALU = mybir.AluOpType

B = 8192          # nodes
D = 128           # feature dim
P = 128           # partitions
NCORES = 8
BC = B // NCORES  # rows per core (1024)
NS = B // P       # 64 j-slices of 128 rows
NQ = BC // P      # 8 q-tiles per core
RG = [list(range(NCORES))]

# threshold shift: t'' = t*(1 - 2^-10) so elements equal to the 33rd value
# land strictly above the threshold (bf16 value gaps are either 0 or
# >= ~2^-9 relative, so the shift never misclassifies rank 34).
SHIFT = 1.0 - 2.0 ** -10


def _r(ap):
    return ap.rearrange("p s d -> p (s d)")


def _nat(dram_ap):
    """DRAM [S*P, D] viewed as SBUF-natural [P, S, D] (row j = s*128+p)."""
    return dram_ap.rearrange("(s p) d -> p s d", p=P)


def _natp(dram_ap, c):
    """p-major DRAM [c*1024, D] viewed as [P, c*8, D]: row r = cb*1024 +
    p*8 + s holds node j = cb*1024 + s*128 + p (per-1024-block p-major).
    8 rows per (partition, block) are contiguous -> 8x fewer DMA
    descriptors than _nat.  The host permutes the features to match;
    purely-internal tensors (dcb/den) just use this view on both sides."""
    return dram_ap.rearrange("(c p s) d -> p c s d", p=P, s=8, c=c)


def build():
    nc = bacc.Bacc(None, target_bir_lowering=False, debug=False)

    feat = nc.declare_dram_parameter("feat", [B, D], F32, isOutput=False)
    featc = nc.declare_dram_parameter("featc", [BC, D], F32, isOutput=False)
    out_ext = nc.declare_dram_parameter("out", [BC, D], F32, isOutput=True)

    with tile.TileContext(nc) as tc:
        with (
            tc.tile_pool(name="dram", bufs=1, space="DRAM") as dr,
            tc.tile_pool(name="pers", bufs=1) as pers,
        ):
            # ---------------- DRAM scratch ----------------
            # scatter bounce in TRANSPOSED layout [8 cores x 128 d, BC j]:
            # block c = St[:, c*BC:(c+1)*BC], so the ReduceScatter hands core
            # c its own j-rows still transposed — no per-[128,128] PE
            # transposes + copies before the DMA, and the merge adds rsT to
            # GT before the single natural-layout transpose pass.
            s_bounce = [
                dr.tile([NCORES * P, BC], BF16, name=f"sbounce{i}") for i in range(2)
            ]
            rs_out = [dr.tile([P, BC], BF16, name=f"rsout{i}") for i in range(2)]
            zct_dram = dr.tile([P, BC], BF16)
            dcb_dram = dr.tile([BC, D], BF16)
            zt_ag = dr.tile([NCORES * P, BC], BF16, addr_space="Shared")
            den_ag = dr.tile([B, D], BF16, addr_space="Shared")
            skew_in = dr.tile([8, 16], BF16)
            skew_out = dr.tile([64, 16], BF16, addr_space="Shared")

            # ---------------- persistent SBUF ----------------
            identf = pers.tile([P, P], F32)
            identb = pers.tile([P, P], BF16)
            ones_col = pers.tile([P, 1], BF16)
            ones1b = pers.tile([1, P], BF16)
            fb32 = pers.tile([P, NS, D], BF16)       # f/32 (gather rhs + phase-8 V)
            fbc32 = pers.tile([P, NQ, D], BF16)      # f_c/32 (scatter-1 stationary)
            xc_nat = pers.tile([P, NQ, D], BF16)     # x_c bf16 (xcT src + z build)
            xcT = pers.tile([P, BC], BF16)           # x_c^T [d, i]
            xbT = pers.tile([P, B], BF16)            # x^T [d, j] (AllGathered)
            tcol = pers.tile([P, NQ], F32)           # raw th (v33+v34) per q-tile
            negtb = pers.tile([1, BC], BF16)         # -t'' per i (rank-1 rhs)
            T_bc = pers.tile([P, BC], BF16)          # +t'' broadcast to all parts
            # natural-layout mask resident in SBUF, fp8 (0/1 exact): row
            # i = q*128+p, all 8192 cols.  Both scatters read it directly —
            # M' never touches DRAM in natural form.
            Mnat = pers.tile([P, NQ, B], FP8)
            # transposed-mask residency: slices [0, NRES) stay in SBUF
            # (NCACHE in the persistent pool, the rest in the diffusion
            # pool); slices [NRES, 64) are recomputed from sim^T in BOTH
            # gathers — that work hides under the concurrent ReduceScatter,
            # while the old DRAM spill contended with it and stalled the PE
            # (HAM re-throttle).
            NCACHE = 24
            NRES = 41
            mtc = [
                pers.tile([P, BC], FP8, name=f"mtc{js}") for js in range(NCACHE)
            ]

            nc.vector.memset(identf[:], 1.0)
            nc.gpsimd.affine_select(
                identf[:], identf[:], pattern=[[1, P]], compare_op=ALU.is_equal,
                fill=0.0, base=0, channel_multiplier=-1,
            )
            nc.vector.memset(identb[:], 1.0)
            nc.gpsimd.affine_select(
                identb[:], identb[:], pattern=[[1, P]], compare_op=ALU.is_equal,
                fill=0.0, base=0, channel_multiplier=-1,
            )
            nc.vector.memset(ones_col[:], 1.0)
            nc.vector.memset(ones1b[:], 1.0)

            # ================ phase 0: load, normalize, layouts ================
            # xb_nat lives in a wrapper pool that outlives p0: the xbT
            # transposes run INSIDE the sim phase (interleaved with q-tile
            # 0's matmuls), so the sim pools no longer wait on the whole of
            # phase 0 -- only on the xb_nat scale-copies they consume.
            _pT_stack = contextlib.ExitStack()
            pT = _pT_stack.enter_context(tc.tile_pool(name="pT", bufs=1))
            xb_nat = pT.tile([P, NS, D], BF16)
            with (
                tc.tile_pool(name="p0", bufs=1) as p0,
                tc.tile_pool(name="ps0", bufs=1, space="PSUM") as ps0,
            ):
                fc_sb = p0.tile([P, NQ, D], F32)
                nc.scalar.dma_start(
                    fc_sb[:].rearrange("p (c s) d -> p c s d", c=1, s=8),
                    _natp(featc[:], 1),
                )
                f_sb = p0.tile([P, NS, D], F32)
                f4 = f_sb[:].rearrange("p (c s) d -> p c s d", c=8, s=8)
                nc.scalar.dma_start(f4[:, 0:4, :, :], _natp(feat[:], 8)[:, 0:4, :, :])
                nc.sync.dma_start(f4[:, 4:, :, :], _natp(feat[:], 8)[:, 4:, :, :])

                # dummy collective: absorbs inter-core launch skew here (its
                # output is never consumed) so the first real collective
                # doesn't pay it
                skw = p0.tile([8, 16], BF16, tag="skw")
                nc.vector.memset(skw[:], 0.0)
                nc.sync.dma_start(skew_in[:], skw[:])
                nc.gpsimd.collective_compute(
                    "AllGather", ALU.bypass, replica_groups=RG,
                    ins=[skew_in[:].opt()], outs=[skew_out[:].opt()],
                )

                # row norms via DVE squares with accumulate
                n2c = p0.tile([P, NQ], F32)
                for q in range(NQ):
                    sq = p0.tile([P, D], F32, tag="sq", bufs=2)
                    nc.vector.scalar_tensor_tensor(
                        sq[:], fc_sb[:, q, :], 1.0, fc_sb[:, q, :],
                        ALU.mult, ALU.mult, accum_out=n2c[:, q:q + 1],
                    )
                nrmc = p0.tile([P, NQ], F32)
                nc.scalar.activation(nrmc[:], n2c[:], AF.Sqrt)
                rnc = p0.tile([P, NQ], F32)
                nc.vector.reciprocal(rnc[:], nrmc[:])
                n2 = p0.tile([P, NS], F32)
                for s in range(NS):
                    sq = p0.tile([P, D], F32, tag="sq", bufs=2)
                    nc.vector.scalar_tensor_tensor(
                        sq[:], f_sb[:, s, :], 1.0, f_sb[:, s, :],
                        ALU.mult, ALU.mult, accum_out=n2[:, s:s + 1],
                    )
                nrm = p0.tile([P, NS], F32)
                nc.scalar.activation(nrm[:], n2[:], AF.Sqrt)
                rn = p0.tile([P, NS], F32)
                nc.vector.reciprocal(rn[:], nrm[:])

                # x_c bf16; xcT via PE transposes
                for q in range(NQ):
                    nc.vector.tensor_scalar(
                        xc_nat[:, q, :], fc_sb[:, q, :], rnc[:, q:q + 1], None,
                        ALU.mult,
                    )
                for qg in range(NQ // 4):
                    psq = ps0.tile([P, 4, P], BF16, tag="ptr", bufs=2)
                    for u in range(4):
                        nc.tensor.transpose(
                            psq[:, u, :], xc_nat[:, qg * 4 + u, :], identb[:]
                        )
                    nc.vector.tensor_copy(
                        xcT[:, qg * 512:(qg + 1) * 512],
                        psq[:].rearrange("p a b -> p (a b)"),
                    )

                # x (all rows) in natural layout; transposed to xbT inside
                # the sim phase
                for s in range(NS):
                    nc.scalar.activation(
                        xb_nat[:, s, :], f_sb[:, s, :], AF.Copy,
                        scale=rn[:, s:s + 1],
                    )

                nc.scalar.activation(_r(fb32[:]), _r(f_sb[:]), AF.Copy, scale=1 / 32)
                nc.scalar.activation(
                    _r(fbc32[:]), _r(fc_sb[:]), AF.Copy, scale=1 / 32
                )

            # ================ phase 2+3: sim, topk threshold, M' ================
            with (
                tc.tile_pool(name="p23", bufs=1) as p23,
                tc.tile_pool(name="ps23", bufs=1, space="PSUM") as psp,
            ):
                # software-pipelined: stage1 sim-mm+copy(qt) | stage2
                # topk(qt-1) | stage3 threshold(qt-2).  Keeps the ACT and
                # DVE queues fed instead of ping-ponging per q-tile.
                simt = {}
                for step in range(NQ + 2):
                    if step < NQ:
                        qt = step
                        simt[qt] = p23.tile([P, 16, 512], BF16, tag="sim",
                                            bufs=3, name=f"sim{qt}")
                        for chp in range(8):
                            if qt == 0:
                                # build xbT on the fly, right before the
                                # chunks that consume it: sim starts without
                                # waiting for the whole transpose pass
                                for ch in (chp * 2, chp * 2 + 1):
                                    psq = psp.tile([P, 4, P], BF16, tag="ptr",
                                                   bufs=2)
                                    for u in range(4):
                                        nc.tensor.transpose(
                                            psq[:, u, :],
                                            xb_nat[:, ch * 4 + u, :],
                                            identb[:],
                                        )
                                    nc.vector.tensor_copy(
                                        xbT[:, ch * 512:(ch + 1) * 512],
                                        psq[:].rearrange("p a b -> p (a b)"),
                                    )
                            pssim = psp.tile([P, 2, 512], F32, tag="pssim",
                                             bufs=3)
                            for u in range(2):
                                ch = chp * 2 + u
                                nc.tensor.matmul(
                                    pssim[:, u, :],
                                    xcT[:, qt * P:(qt + 1) * P],
                                    xbT[:, ch * 512:(ch + 1) * 512],
                                    start=True, stop=True,
                                )
                            dst = simt[qt][:].rearrange("p c f -> p (c f)")[
                                :, chp * 1024:(chp + 1) * 1024
                            ]
                            src = pssim[:].rearrange("p c f -> p (c f)")
                            if chp % 4 != 3:
                                nc.scalar.activation(dst, src, AF.Copy)
                            else:
                                nc.vector.tensor_copy(dst, src)
                    if 1 <= step <= NQ:
                        qt = step - 1
                        simf = simt[qt][:].rearrange("p c f -> p (c f)")
                        cand = p23.tile([P, 8, 8], BF16, tag="cand", bufs=2)
                        for c in range(8):
                            nc.vector.max(
                                cand[:, c, :], simf[:, c * 1024:(c + 1) * 1024]
                            )
                        candf = cand[:].rearrange("p c f -> p (c f)")
                        m8 = None
                        for rnd in range(5):
                            m8 = p23.tile([P, 8], BF16, tag="m8", bufs=6)
                            nc.vector.max(m8[:], candf)
                            if rnd < 4:
                                nc.vector.match_replace(
                                    candf, m8[:], candf, -1e30
                                )
                        th = p23.tile([P, 1], F32, tag="th", bufs=2)
                        nc.vector.tensor_tensor(
                            th[:], m8[:, 0:1], m8[:, 1:2], ALU.add
                        )
                        nc.vector.tensor_copy(tcol[:, qt:qt + 1], th[:])
                    if step >= 2:
                        qt = step - 2
                        simf = simt.pop(qt)[:].rearrange("p c f -> p (c f)")
                        tneg = p23.tile([P, 1], F32, tag="tneg", bufs=2)
                        nc.vector.tensor_scalar(
                            tneg[:], tcol[:, qt:qt + 1], -0.5e9 * SHIFT,
                            None, ALU.mult,
                        )
                        for hv in range(2):
                            nc.scalar.activation(
                                Mnat[:, qt, hv * 4096:(hv + 1) * 4096],
                                simf[:, hv * 4096:(hv + 1) * 4096],
                                AF.Sigmoid, bias=tneg[:], scale=1e9,
                            )

            # xb_nat (wrapper pool) is dead once the last xbT transpose ran
            _pT_stack.close()

            # ---- threshold row layout: negtb [1, BC], T_bc [P, BC] ----
            with (
                tc.tile_pool(name="throw", bufs=1) as trw,
                tc.tile_pool(name="pst", bufs=1, space="PSUM") as pst,
            ):
                tposb = trw.tile([1, BC], BF16)
                for q in range(NQ):
                    ps1 = pst.tile([1, P], F32, tag="t1", bufs=2)
                    nc.tensor.transpose(ps1[:], tcol[:, q:q + 1], identf[:])
                    nc.vector.tensor_scalar(
                        negtb[0:1, q * P:(q + 1) * P], ps1[:],
                        -0.5 * SHIFT, None, ALU.mult,
                    )
                    nc.vector.tensor_scalar(
                        tposb[0:1, q * P:(q + 1) * P], ps1[:],
                        0.5 * SHIFT, None, ALU.mult,
                    )
                for h in range(2):
                    psb = pst.tile([P, 512], F32, tag="tb", bufs=2)
                    nc.tensor.matmul(
                        psb[:], ones1b[:], tposb[0:1, h * 512:(h + 1) * 512],
                        start=True, stop=True,
                    )
                    nc.scalar.activation(
                        T_bc[:, h * 512:(h + 1) * 512], psb[:], AF.Copy
                    )

            # ================ diffusion (x2) ================
            def diffusion(i, dpool, rhsc_t, den_t, den_src, csrc, cscale,
                          gscale=1.0, do_ag=True):
                """one step: returns dc = (A @ src)_rows-of-core (fp32).

                rhsc_t [P,NQ,D] bf16: src_c/32 (scatter stationary)
                den_t  [P,NS,D] bf16: src/32 (or src; gather stationary)
                den_src: if not None, DRAM ap to load den_t from (after the
                         scatter is issued; sync queue so nothing blocks)
                csrc/cscale: merge-time correction, adds cscale*csrc
                """
                # ---- scatter: St[d, j] = sum_i (src_c/32)[i, d] * M'[i, j]
                # M' comes straight from SBUF (Mnat) — no DMA at all.
                # St stays transposed [d, j]: core block c = cols
                # [c*BC, (c+1)*BC) DMAs straight to bounce rows
                # [c*P, (c+1)*P) — no transposes before the collective.
                S_sbT = None
                with tc.tile_pool(name=f"pscat{i}", bufs=1, space="PSUM") as psc:
                    for half in range(2):
                        S_sbT = dpool.tile([P, 8, 512], BF16, tag="Ssb",
                                           bufs=2)
                        psSt = [
                            psc.tile([P, 512], F32, tag="acc", bufs=8,
                                     name=f"psSt{i}_{half}_{js}")
                            for js in range(8)
                        ]
                        for q in range(NQ):
                            for js in range(8):
                                c0 = half * 4096 + js * 512
                                nc.tensor.matmul(
                                    psSt[js][:], rhsc_t[:, q, :],
                                    Mnat[:, q, c0:c0 + 512],
                                    start=(q == 0), stop=(q == NQ - 1),
                                )
                        for js in range(8):
                            if js % 2 == 0:
                                nc.scalar.activation(
                                    S_sbT[:, js, :], psSt[js][:], AF.Copy
                                )
                            else:
                                nc.vector.tensor_copy(
                                    S_sbT[:, js, :], psSt[js][:]
                                )
                        # this half's 4 core-blocks fly while the other half
                        # is still accumulating
                        # all on the scalar queue: the sync queue must stay
                        # clear so diffusion-2's den load fires the moment
                        # the AllGather lands
                        for cb in range(4):
                            c = half * 4 + cb
                            eng = nc.scalar
                            eng.dma_start(
                                s_bounce[i][c * P:(c + 1) * P, :],
                                S_sbT[:, cb * 2:(cb + 1) * 2, :]
                                .rearrange("p a b -> p (a b)"),
                            )
                nc.gpsimd.collective_compute(
                    "ReduceScatter", ALU.add, replica_groups=RG,
                    ins=[s_bounce[i][:].opt()], outs=[rs_out[i][:].opt()],
                )

                if den_src is not None:
                    nc.sync.dma_start(
                        den_t[:].rearrange("p (c s) d -> p c s d", c=8, s=8),
                        _natp(den_src[:], 8),
                    )

                # ---- gather: Gt[d, i] = sum_j src[j, d] * M'[i, j]
                # pass 0: M'^T j-slices recomputed from sim^T = xbT_js^T @ xcT
                # (fp8, exact for a 0/1 mask), all 64 kept in SBUF; pass 1
                # re-reads them with no DMA at all.
                with tc.tile_pool(name=f"pgat{i}", bufs=1, space="PSUM") as psg:
                    psGt = [
                        psg.tile([P, 512], F32, tag="gacc", bufs=2,
                                 name=f"psGt{i}_{h}")
                        for h in range(2)
                    ]
                    for js in range(NS):
                        if js < NCACHE:
                            MT = mtc[js]
                        elif js < NRES:
                            MT = mtd[js - NCACHE]
                        else:
                            MT = dpool.tile([P, BC], FP8, tag="MTr", bufs=2)
                        if i == 0 or js >= NRES:
                            for h in range(2):
                                rg = psg.tile([P, 512], F32, tag="rg", bufs=6)
                                if js % 2 == 0:
                                    nc.tensor.matmul(
                                        rg[:], xbT[:, js * P:(js + 1) * P],
                                        xcT[:, h * 512:(h + 1) * 512],
                                        start=True, stop=False,
                                    )
                                    nc.tensor.matmul(
                                        rg[:], ones1b[:],
                                        negtb[0:1, h * 512:(h + 1) * 512],
                                        start=False, stop=True,
                                    )
                                    nc.scalar.activation(
                                        MT[:, h * 512:(h + 1) * 512], rg[:],
                                        AF.Sigmoid, scale=1e9,
                                    )
                                else:
                                    nc.tensor.matmul(
                                        rg[:], xbT[:, js * P:(js + 1) * P],
                                        xcT[:, h * 512:(h + 1) * 512],
                                        start=True, stop=True,
                                    )
                                    nc.vector.tensor_tensor(
                                        MT[:, h * 512:(h + 1) * 512], rg[:],
                                        T_bc[:, h * 512:(h + 1) * 512],
                                        ALU.is_ge,
                                    )
                        for h in range(2):
                            nc.tensor.matmul(
                                psGt[h][:], den_t[:, js, :],
                                MT[:, h * 512:(h + 1) * 512],
                                start=(js == 0), stop=(js == NS - 1),
                            )
                    # keep the merge (which waits on the ReduceScatter) from
                    # being scheduled into the js loop's queues — a slow
                    # collective at a queue head stalls every engine
                    tc.no_sync_barrier()
                    GT = dpool.tile([P, 2, 512], F32, tag="GT", bufs=1)
                    for h in range(2):
                        nc.scalar.activation(GT[:, h, :], psGt[h][:], AF.Copy)
                    GTf = GT[:].rearrange("p a b -> p (a b)")

                    # merge rs (already transposed [d, j]) with G^T BEFORE
                    # the transpose pass: one STT + 8 transposes total
                    rsT = dpool.tile([P, BC], BF16, tag="rsT", bufs=1)
                    nc.sync.dma_start(rsT[:], rs_out[i][:])
                    tmpT = dpool.tile([P, BC], BF16, tag="tmpT", bufs=1)
                    nc.vector.scalar_tensor_tensor(
                        tmpT[:], GTf, gscale, rsT[:], ALU.mult, ALU.add,
                    )
                    dc = dpool.tile([P, NQ, D], F32, tag=f"dc{i}")
                    for q in range(NQ):
                        psb = psg.tile([P, P], BF16, tag="rg", bufs=6,
                                       name=f"ptrG{i}_{q}")
                        nc.tensor.transpose(
                            psb[:], tmpT[:, q * P:(q + 1) * P], identb[:]
                        )
                        nc.vector.scalar_tensor_tensor(
                            dc[:, q, :], csrc[:, q, :], cscale, psb[:],
                            ALU.mult, ALU.add,
                        )
                if do_ag:
                    dcb = dpool.tile([P, NQ, D], BF16, tag="dcb", bufs=1)
                    nc.scalar.activation(_r(dcb[:]), _r(dc[:]), AF.Copy)
                    nc.scalar.dma_start(
                        _natp(dcb_dram[:], 1),
                        dcb[:].rearrange("p (c s) d -> p c s d", c=1, s=8),
                    )
                    nc.gpsimd.collective_compute(
                        "AllGather", ALU.bypass, replica_groups=RG,
                        ins=[dcb_dram[:].opt()], outs=[den_ag[:].opt()],
                    )
                return dc

            with tc.tile_pool(name="dif", bufs=1) as dpool:
                # middle of the transposed mask (slices NCACHE..NRES-1),
                # allocated here so it reuses the sim pools' freed SBUF
                mtd = [
                    dpool.tile([P, BC], FP8, name=f"mtd{js}")
                    for js in range(NRES - NCACHE)
                ]
                dc1 = diffusion(0, dpool, fbc32, fb32, None, fbc32, -2.0)
                # operands for pass 2 (diff1 arrives bf16 via AllGather);
                # den2 stays unscaled, the gather merge divides by 32.
                rhsc2 = dpool.tile([P, NQ, D], BF16, tag="rhsc2")
                nc.scalar.activation(_r(rhsc2[:]), _r(dc1[:]), AF.Copy, scale=1 / 32)
                den2 = dpool.tile([P, NS, D], BF16, tag="den2")

                dc2 = diffusion(1, dpool, rhsc2, den2, den_ag, dc1, -1.0 / 16.0,
                                gscale=1.0 / 32.0, do_ag=False)

                # ---- phase 7: z_c = x_c + 0.1 * geo_c/||geo_c||; AllGather
                # z^T in two column-halves so phase 8 can start scoring the
                # first half while the second is still in flight.  Fully
                # per-q pipelined: no whole-tile sqrt/recip barrier.
                n2g = dpool.tile([P, NQ], F32)
                ng = dpool.tile([P, NQ], F32)
                rg01 = dpool.tile([P, NQ], F32)
                zbc = dpool.tile([P, NQ, D], BF16)
                zcT = dpool.tile([P, BC], BF16, tag="zcT")
                with tc.tile_pool(name="psz", bufs=1, space="PSUM") as psz:
                    for q in range(NQ):
                        sq = dpool.tile([P, D], F32, tag="sqg", bufs=2)
                        nc.vector.scalar_tensor_tensor(
                            sq[:], dc2[:, q, :], 1.0, dc2[:, q, :],
                            ALU.mult, ALU.mult, accum_out=n2g[:, q:q + 1],
                        )
                        nc.scalar.activation(
                            ng[:, q:q + 1], n2g[:, q:q + 1], AF.Sqrt
                        )
                        rgn = dpool.tile([P, 1], F32, tag="rgn", bufs=2)
                        nc.vector.reciprocal(rgn[:], ng[:, q:q + 1])
                        nc.vector.tensor_scalar(
                            rg01[:, q:q + 1], rgn[:], 0.1, None, ALU.mult
                        )
                        nc.vector.scalar_tensor_tensor(
                            zbc[:, q, :], dc2[:, q, :], rg01[:, q:q + 1],
                            xc_nat[:, q, :], ALU.mult, ALU.add,
                        )
                        psq = psz.tile([P, P], BF16, tag="ptr", bufs=2)
                        nc.tensor.transpose(psq[:], zbc[:, q, :], identb[:])
                        if q % 2 == 0:
                            nc.scalar.activation(
                                zcT[:, q * P:(q + 1) * P], psq[:], AF.Copy
                            )
                        else:
                            nc.vector.tensor_copy(
                                zcT[:, q * P:(q + 1) * P], psq[:]
                            )
                nc.scalar.dma_start(zct_dram[:], zcT[:])
                nc.gpsimd.collective_compute(
                    "AllGather", ALU.bypass, replica_groups=RG,
                    ins=[zct_dram[:].opt()], outs=[zt_ag[:].opt()],
                )

            # ================ phase 8: softmax attention ================
            with (
                tc.tile_pool(name="p8", bufs=1) as p8,
                tc.tile_pool(name="ps8", bufs=1, space="PSUM") as psp,
            ):
                zT = p8.tile([P, B], BF16)
                for c in range(NCORES):
                    eng = (nc.sync, nc.scalar, nc.gpsimd)[c % 3]
                    eng.dma_start(
                        zT[:, c * BC:(c + 1) * BC],
                        zt_ag[c * P:(c + 1) * P, :],
                    )
                jc_list = list(range(NS))
                # keep the PE HAM window busy while the z AllGather lands so
                # the score matmuls start at 2.4 GHz, not 1.2.  Real matmuls,
                # not transposes — transpose-mode does not count as PE-busy
                # for the HAM activity window.
                wsc = p8.tile([P, P], F32, tag="warm")
                for w in range(48):
                    psw = psp.tile([P, P], F32, tag="psw", bufs=1)
                    nc.tensor.matmul(
                        psw[:], identb[:], identb[:], start=True, stop=True,
                    )
                    if w == 47:
                        nc.vector.tensor_copy(wsc[:], psw[:])
                for grp in range(2):
                    psOUT = psp.tile([P, 512], F32, tag="psOUT", bufs=1)
                    psS1 = psp.tile([1, 512], F32, tag="psS1", bufs=1)
                    sacc = p8.tile([P, 512], F32, tag="sacc", bufs=2)
                    nc.vector.memset(sacc[:], 0.0)
                    # software-pipelined: scores(jp) | exp(jp-1) | V+denom
                    # accumulate(jp-2).  Denominator split: slice 0 on PE
                    # (ones matmul), slice 1 on DVE (sacc accumulate).
                    psTt, Ptt = {}, {}
                    for step in range(NS // 2 + 2):
                        if step < NS // 2:
                            jp = step
                            psTt[jp] = psp.tile([P, 2, 512], F32, tag="psT",
                                                bufs=2, name=f"psT{grp}_{jp}")
                            for u in range(2):
                                jc = jc_list[jp * 2 + u]
                                nc.tensor.matmul(
                                    psTt[jp][:, u, :],
                                    zT[:, jc * P:(jc + 1) * P],
                                    xcT[:, grp * 512:(grp + 1) * 512],
                                    start=True, stop=True,
                                )
                        if 1 <= step <= NS // 2:
                            jp = step - 1
                            Ptt[jp] = p8.tile([P, 2, 512], BF16, tag="Pt",
                                              bufs=4, name=f"Pt{grp}_{jp}")
                            psT = psTt.pop(jp)
                            nc.scalar.activation(
                                Ptt[jp][:].rearrange("p a b -> p (a b)"),
                                psT[:].rearrange("p a b -> p (a b)"),
                                AF.Exp, scale=10.0,
                            )
                        if step >= 2:
                            jp = step - 2
                            Pt = Ptt.pop(jp)
                            for u in range(2):
                                pos = jp * 2 + u
                                jc = jc_list[pos]
                                nc.tensor.matmul(
                                    psOUT[:], fb32[:, jc, :], Pt[:, u, :],
                                    start=(pos == 0), stop=(pos == NS - 1),
                                )
                            nc.tensor.matmul(
                                psS1[:], ones_col[:], Pt[:, 0, :],
                                start=(jp == 0), stop=(jp == NS // 2 - 1),
                            )
                            nc.vector.scalar_tensor_tensor(
                                sacc[:], Pt[:, 1, :], 1.0, sacc[:],
                                ALU.mult, ALU.add,
                            )
                    # fold the PE partial into sacc row 0, then reduce over
                    # partitions via 4 PE transposes
                    nc.vector.tensor_tensor(
                        sacc[0:1, :], sacc[0:1, :], psS1[:], ALU.add,
                    )
                    # denom: reduce sacc over partitions via 4 PE transposes
                    s1nat = p8.tile([P, 4], F32, tag="s1nat", bufs=2)
                    scr = p8.tile([P, P], F32, tag="scr", bufs=2)
                    for b in range(4):
                        psB = psp.tile([P, P], F32, tag="psB", bufs=1)
                        nc.tensor.transpose(
                            psB[:], sacc[:, b * P:(b + 1) * P], identf[:]
                        )
                        nc.vector.tensor_scalar(
                            scr[:], psB[:], 1.0, 0.0, ALU.mult, ALU.add,
                            accum_out=s1nat[:, b:b + 1],
                        )
                    rnat = p8.tile([P, 4], F32, tag="rnat", bufs=2)
                    nc.vector.reciprocal(rnat[:], s1nat[:])
                    rnat32 = p8.tile([P, 4], F32, tag="rnat32", bufs=2)
                    nc.vector.tensor_scalar(
                        rnat32[:], rnat[:], 32.0, None, ALU.mult
                    )
                    OUT_sb = p8.tile([P, 512], F32, tag="OUTsb", bufs=2)
                    nc.scalar.activation(OUT_sb[:], psOUT[:], AF.Copy)
                    for b in range(4):
                        psB = psp.tile([P, P], F32, tag="psB", bufs=1)
                        nc.tensor.transpose(
                            psB[:], OUT_sb[:, b * P:(b + 1) * P], identf[:]
                        )
                        ob = p8.tile([P, D], F32, tag="ob", bufs=2)
                        nc.vector.tensor_scalar(
                            ob[:], psB[:], rnat32[:, b:b + 1], None, ALU.mult
                        )
                        nc.scalar.dma_start(
                            out_ext[grp * 512 + b * P: grp * 512 + (b + 1) * P, :],
                            ob[:],
                        )

    nc.finalize()
    return nc


_NC_CACHE = None


def kernel(features: np.ndarray) -> np.ndarray:
    global _NC_CACHE
    features = np.ascontiguousarray(np.asarray(features, np.float32))
    assert features.shape == (B, D), features.shape
    if _NC_CACHE is None:
        _NC_CACHE = build()
    # per-1024-block p-major relabeling: device row c*1024 + p*8 + s holds
    # node c*1024 + s*128 + p, so every [N*128, D] DMA runs with 4 KB
    # contiguous descriptors.  The computation is permutation-equivariant;
    # device output rows come back in natural node order (out writes are
    # natural-layout), so no un-permute is needed.
    feat_dev = np.ascontiguousarray(
        features.reshape(NCORES, 8, P, D).swapaxes(1, 2).reshape(B, D)
    )
    in_maps = [
        {
            "feat": feat_dev,
            "featc": feat_dev[c * BC:(c + 1) * BC].copy(),
        }
        for c in range(NCORES)
    ]
    res = run_bass_kernel_spmd(_NC_CACHE, in_maps, core_ids=list(range(NCORES)))
    return np.concatenate(
        [np.asarray(res.results[c]["out"], np.float32) for c in range(NCORES)],
        axis=0,
    )



# revision 50
# speedup vs baseline: 1.2132x; 1.2132x over previous
"""Trainium2 Bass kernel for nn_ALMSLayer (gnn_message_passing), 8 NeuronCores.

Algorithm (per core c, rows R_c = [c*1024, (c+1)*1024) of B=8192):
  x       = f / ||f||                      (rows normalized)
  sim     = x_c @ x^T                      (bf16 matmul, [1024, 8192])
  topk    : per row, 33rd/34th-largest via chunked top-8 candidates ->
            threshold t; M' = (sim >= t) in {0,1} (includes self edge)
  A       = (M' + M'^T)/32 - I/16          (-I/16 corrects the self edges)
  diff1   = A @ f ; geodesic = A @ diff1   (dense bf16 matmuls; M'^T side
            summed across cores with ReduceScatter, M' side local, full
            tensors rebuilt with a bf16 AllGather)
  z       = x + 0.1 * geodesic/||geodesic||
  out     = softmax((x_c @ z^T)/0.1) @ f   (flash-attention style)

Key perf structure vs the naive layout:
  * Masks never touch DRAM.  M' (natural, fp8) lives in SBUF; M'^T j-slices
    are recomputed on the fly from sim^T = xbT_js^T @ xcT (one PE matmul)
    with the per-i threshold applied by a rank-1 PSUM accumulate + sigmoid
    (even slices) or a broadcast-threshold compare on DVE (odd); slices
    [0, NRES) are kept resident, the tail is recomputed again in gather-2
    where it hides under the concurrent ReduceScatter.
  * The scatter bounce is kept TRANSPOSED [8*128 d, BC j] so ReduceScatter
    block c needs no pre-transpose; rs^T merges into G^T before a single
    8-transpose pass back to natural layout.
  * All [N*128, D] feature DMAs use a per-1024-block p-major DRAM layout
    (host pre-permutes; labels cancel so the output needs no un-permute),
    giving 4 KB contiguous descriptors instead of 256-512 B.
  * x^T and z^T full tensors are built by AllGathering locally PE-transposed
    [D, 1024] chunks (natural-layout loads), never by DMA transpose.
  * diff2's scatter is issued before diff1's AllGather completes; the sync
    DMA queue is kept clear of bounce writes so diffusion-2's den load
    fires the moment the AllGather lands.
  * Phase 8 warms the HAM clock gate with real matmuls (transpose-mode
    does not count as PE activity) so scoring starts at 2.4 GHz.

Host side only shards/permutes/replicates/concats; all arithmetic on device.
"""
import contextlib
import sys

sys.path.insert(0, "/opt/trn_rl_repo")

import numpy as np

import concourse.bass as bass
import concourse.tile as tile
from concourse import bacc, mybir
from concourse.bass_utils import run_bass_kernel_spmd

F32 = mybir.dt.float32
BF16 = mybir.dt.bfloat16
FP8 = mybir.dt.float8e4
AF = mybir.ActivationFunctionType
ALU = mybir.AluOpType

B = 8192          # nodes
D = 128           # feature dim
P = 128           # partitions
NCORES = 8
BC = B // NCORES  # rows per core (1024)
NS = B // P       # 64 j-slices of 128 rows
NQ = BC // P      # 8 q-tiles per core
RG = [list(range(NCORES))]

# threshold shift: t'' = t*(1 - 2^-10) so elements equal to the 33rd value
# land strictly above the threshold (bf16 value gaps are either 0 or
# >= ~2^-9 relative, so the shift never misclassifies rank 34).
SHIFT = 1.0 - 2.0 ** -10


def _r(ap):
    return ap.rearrange("p s d -> p (s d)")


def _nat(dram_ap):
    """DRAM [S*P, D] viewed as SBUF-natural [P, S, D] (row j = s*128+p)."""
    return dram_ap.rearrange("(s p) d -> p s d", p=P)


def _natp(dram_ap, c):
    """p-major DRAM [c*1024, D] viewed as [P, c*8, D]: row r = cb*1024 +
    p*8 + s holds node j = cb*1024 + s*128 + p (per-1024-block p-major).
    8 rows per (partition, block) are contiguous -> 8x fewer DMA
    descriptors than _nat.  The host permutes the features to match;
    purely-internal tensors (dcb/den) just use this view on both sides."""
    return dram_ap.rearrange("(c p s) d -> p c s d", p=P, s=8, c=c)


def build():
    nc = bacc.Bacc(None, target_bir_lowering=False, debug=False)

    feat = nc.declare_dram_parameter("feat", [B, D], F32, isOutput=False)
    featc = nc.declare_dram_parameter("featc", [BC, D], F32, isOutput=False)
    out_ext = nc.declare_dram_parameter("out", [BC, D], F32, isOutput=True)

    with tile.TileContext(nc) as tc:
        with (
            tc.tile_pool(name="dram", bufs=1, space="DRAM") as dr,
            tc.tile_pool(name="pers", bufs=1) as pers,
        ):
            # ---------------- DRAM scratch ----------------
            # scatter bounce in TRANSPOSED layout [8 cores x 128 d, BC j]:
            # block c = St[:, c*BC:(c+1)*BC], so the ReduceScatter hands core
            # c its own j-rows still transposed — no per-[128,128] PE
            # transposes + copies before the DMA, and the merge adds rsT to
            # GT before the single natural-layout transpose pass.
            s_bounce = [
                dr.tile([NCORES * P, BC], BF16, name=f"sbounce{i}") for i in range(2)
            ]
            rs_out = [dr.tile([P, BC], BF16, name=f"rsout{i}") for i in range(2)]
            zct_dram = dr.tile([P, BC], BF16)
            dcb_dram = dr.tile([BC, D], BF16)
            zt_ag = dr.tile([NCORES * P, BC], BF16, addr_space="Shared")
            den_ag = dr.tile([B, D], BF16, addr_space="Shared")
            skew_in = dr.tile([8, 16], BF16)
            skew_out = dr.tile([64, 16], BF16, addr_space="Shared")

            # ---------------- persistent SBUF ----------------
            identf = pers.tile([P, P], F32)
            identb = pers.tile([P, P], BF16)
            ones_col = pers.tile([P, 1], BF16)
            ones1b = pers.tile([1, P], BF16)
            fb32 = pers.tile([P, NS, D], BF16)       # f/32 (gather rhs + phase-8 V)
            fbc32 = pers.tile([P, NQ, D], BF16)      # f_c/32 (scatter-1 stationary)
            xc_nat = pers.tile([P, NQ, D], BF16)     # x_c bf16 (xcT src + z build)
            xcT = pers.tile([P, BC], BF16)           # x_c^T [d, i]
            xbT = pers.tile([P, B], BF16)            # x^T [d, j] (AllGathered)
            tcol = pers.tile([P, NQ], F32)           # raw th (v33+v34) per q-tile
            negtb = pers.tile([1, BC], BF16)         # -t'' per i (rank-1 rhs)
            T_bc = pers.tile([P, BC], BF16)          # +t'' broadcast to all parts
            # natural-layout mask resident in SBUF, fp8 (0/1 exact): row
            # i = q*128+p, all 8192 cols.  Both scatters read it directly —
            # M' never touches DRAM in natural form.
            Mnat = pers.tile([P, NQ, B], FP8)
            # transposed-mask residency: slices [0, NRES) stay in SBUF
            # (NCACHE in the persistent pool, the rest in the diffusion
            # pool); slices [NRES, 64) are recomputed from sim^T in BOTH
            # gathers — that work hides under the concurrent ReduceScatter,
            # while the old DRAM spill contended with it and stalled the PE
            # (HAM re-throttle).
            NCACHE = 24
            NRES = 41
            mtc = [
                pers.tile([P, BC], FP8, name=f"mtc{js}") for js in range(NCACHE)
            ]

            nc.vector.memset(identf[:], 1.0)
            nc.gpsimd.affine_select(
                identf[:], identf[:], pattern=[[1, P]], compare_op=ALU.is_equal,
                fill=0.0, base=0, channel_multiplier=-1,
            )
            nc.vector.memset(identb[:], 1.0)
            nc.gpsimd.affine_select(
                identb[:], identb[:], pattern=[[1, P]], compare_op=ALU.is_equal,
                fill=0.0, base=0, channel_multiplier=-1,
            )
            nc.vector.memset(ones_col[:], 1.0)
            nc.vector.memset(ones1b[:], 1.0)

            # ================ phase 0: load, normalize, layouts ================
            # xb_nat lives in a wrapper pool that outlives p0: the xbT
            # transposes run INSIDE the sim phase (interleaved with q-tile
            # 0's matmuls), so the sim pools no longer wait on the whole of
            # phase 0 -- only on the xb_nat scale-copies they consume.
            _pT_stack = contextlib.ExitStack()
            pT = _pT_stack.enter_context(tc.tile_pool(name="pT", bufs=1))
            xb_nat = pT.tile([P, NS, D], BF16)
            with (
                tc.tile_pool(name="p0", bufs=1) as p0,
                tc.tile_pool(name="ps0", bufs=1, space="PSUM") as ps0,
            ):
                fc_sb = p0.tile([P, NQ, D], F32)
                nc.scalar.dma_start(
                    fc_sb[:].rearrange("p (c s) d -> p c s d", c=1, s=8),
                    _natp(featc[:], 1),
                )
                f_sb = p0.tile([P, NS, D], F32)
                f4 = f_sb[:].rearrange("p (c s) d -> p c s d", c=8, s=8)
                nc.scalar.dma_start(f4[:, 0:4, :, :], _natp(feat[:], 8)[:, 0:4, :, :])
                nc.sync.dma_start(f4[:, 4:, :, :], _natp(feat[:], 8)[:, 4:, :, :])

                # dummy collective: absorbs inter-core launch skew here (its
                # output is never consumed) so the first real collective
                # doesn't pay it
                skw = p0.tile([8, 16], BF16, tag="skw")
                nc.vector.memset(skw[:], 0.0)
                nc.sync.dma_start(skew_in[:], skw[:])
                nc.gpsimd.collective_compute(
                    "AllGather", ALU.bypass, replica_groups=RG,
                    ins=[skew_in[:].opt()], outs=[skew_out[:].opt()],
                )

                # row norms via DVE squares with accumulate
                n2c = p0.tile([P, NQ], F32)
                for q in range(NQ):
                    sq = p0.tile([P, D], F32, tag="sq", bufs=2)
                    nc.vector.scalar_tensor_tensor(
                        sq[:], fc_sb[:, q, :], 1.0, fc_sb[:, q, :],
                        ALU.mult, ALU.mult, accum_out=n2c[:, q:q + 1],
                    )
                nrmc = p0.tile([P, NQ], F32)
                nc.scalar.activation(nrmc[:], n2c[:], AF.Sqrt)
                rnc = p0.tile([P, NQ], F32)
                nc.vector.reciprocal(rnc[:], nrmc[:])
                n2 = p0.tile([P, NS], F32)
                for s in range(NS):
                    sq = p0.tile([P, D], F32, tag="sq", bufs=2)
                    nc.vector.scalar_tensor_tensor(
                        sq[:], f_sb[:, s, :], 1.0, f_sb[:, s, :],
                        ALU.mult, ALU.mult, accum_out=n2[:, s:s + 1],
                    )
                nrm = p0.tile([P, NS], F32)
                nc.scalar.activation(nrm[:], n2[:], AF.Sqrt)
                rn = p0.tile([P, NS], F32)
                nc.vector.reciprocal(rn[:], nrm[:])

                # x_c bf16; xcT via PE transposes
                for q in range(NQ):
                    nc.vector.tensor_scalar(
                        xc_nat[:, q, :], fc_sb[:, q, :], rnc[:, q:q + 1], None,
                        ALU.mult,
                    )
                for qg in range(NQ // 4):
                    psq = ps0.tile([P, 4, P], BF16, tag="ptr", bufs=2)
                    for u in range(4):
                        nc.tensor.transpose(
                            psq[:, u, :], xc_nat[:, qg * 4 + u, :], identb[:]
                        )
                    nc.vector.tensor_copy(
                        xcT[:, qg * 512:(qg + 1) * 512],
                        psq[:].rearrange("p a b -> p (a b)"),
                    )

                # x (all rows) in natural layout; transposed to xbT inside
                # the sim phase
                for s in range(NS):
                    nc.scalar.activation(
                        xb_nat[:, s, :], f_sb[:, s, :], AF.Copy,
                        scale=rn[:, s:s + 1],
                    )

                nc.scalar.activation(_r(fb32[:]), _r(f_sb[:]), AF.Copy, scale=1 / 32)
                nc.scalar.activation(
                    _r(fbc32[:]), _r(fc_sb[:]), AF.Copy, scale=1 / 32
                )

            # ================ phase 2+3: sim, topk threshold, M' ================
            with (
                tc.tile_pool(name="p23", bufs=1) as p23,
                tc.tile_pool(name="ps23", bufs=1, space="PSUM") as psp,
            ):
                # software-pipelined: stage1 sim-mm+copy(qt) | stage2
                # topk(qt-1) | stage3 threshold(qt-2).  Keeps the ACT and
                # DVE queues fed instead of ping-ponging per q-tile.
                simt = {}
                for step in range(NQ + 2):
                    if step < NQ:
                        qt = step
                        simt[qt] = p23.tile([P, 16, 512], BF16, tag="sim",
                                            bufs=3, name=f"sim{qt}")
                        for chp in range(8):
                            if qt == 0:
                                # build xbT on the fly, right before the
                                # chunks that consume it: sim starts without
                                # waiting for the whole transpose pass
                                for ch in (chp * 2, chp * 2 + 1):
                                    psq = psp.tile([P, 4, P], BF16, tag="ptr",
                                                   bufs=2)
                                    for u in range(4):
                                        nc.tensor.transpose(
                                            psq[:, u, :],
                                            xb_nat[:, ch * 4 + u, :],
                                            identb[:],
                                        )
                                    nc.vector.tensor_copy(
                                        xbT[:, ch * 512:(ch + 1) * 512],
                                        psq[:].rearrange("p a b -> p (a b)"),
                                    )
                            pssim = psp.tile([P, 2, 512], F32, tag="pssim",
                                             bufs=3)
                            for u in range(2):
                                ch = chp * 2 + u
                                nc.tensor.matmul(
                                    pssim[:, u, :],
                                    xcT[:, qt * P:(qt + 1) * P],
                                    xbT[:, ch * 512:(ch + 1) * 512],
                                    start=True, stop=True,
                                )
                            dst = simt[qt][:].rearrange("p c f -> p (c f)")[
                                :, chp * 1024:(chp + 1) * 1024
                            ]
                            src = pssim[:].rearrange("p c f -> p (c f)")
                            if chp % 4 != 3:
                                nc.scalar.activation(dst, src, AF.Copy)
                            else:
                                nc.vector.tensor_copy(dst, src)
                    if 1 <= step <= NQ:
                        qt = step - 1
                        simf = simt[qt][:].rearrange("p c f -> p (c f)")
                        cand = p23.tile([P, 8, 8], BF16, tag="cand", bufs=2)
                        for c in range(8):
                            nc.vector.max(
                                cand[:, c, :], simf[:, c * 1024:(c + 1) * 1024]
                            )
                        candf = cand[:].rearrange("p c f -> p (c f)")
                        m8 = None
                        for rnd in range(5):
                            m8 = p23.tile([P, 8], BF16, tag="m8", bufs=6)
                            nc.vector.max(m8[:], candf)
                            if rnd < 4:
                                nc.vector.match_replace(
                                    candf, m8[:], candf, -1e30
                                )
                        th = p23.tile([P, 1], F32, tag="th", bufs=2)
                        nc.vector.tensor_tensor(
                            th[:], m8[:, 0:1], m8[:, 1:2], ALU.add
                        )
                        nc.vector.tensor_copy(tcol[:, qt:qt + 1], th[:])
                    if step >= 2:
                        qt = step - 2
                        simf = simt.pop(qt)[:].rearrange("p c f -> p (c f)")
                        tneg = p23.tile([P, 1], F32, tag="tneg", bufs=2)
                        nc.vector.tensor_scalar(
                            tneg[:], tcol[:, qt:qt + 1], -0.5e9 * SHIFT,
                            None, ALU.mult,
                        )
                        for hv in range(2):
                            nc.scalar.activation(
                                Mnat[:, qt, hv * 4096:(hv + 1) * 4096],
                                simf[:, hv * 4096:(hv + 1) * 4096],
                                AF.Sigmoid, bias=tneg[:], scale=1e9,
                            )

            # xb_nat (wrapper pool) is dead once the last xbT transpose ran
            _pT_stack.close()

            # ---- threshold row layout: negtb [1, BC], T_bc [P, BC] ----
            with (
                tc.tile_pool(name="throw", bufs=1) as trw,
                tc.tile_pool(name="pst", bufs=1, space="PSUM") as pst,
            ):
                tposb = trw.tile([1, BC], BF16)
                for q in range(NQ):
                    ps1 = pst.tile([1, P], F32, tag="t1", bufs=2)
                    nc.tensor.transpose(ps1[:], tcol[:, q:q + 1], identf[:])
                    nc.vector.tensor_scalar(
                        negtb[0:1, q * P:(q + 1) * P], ps1[:],
                        -0.5 * SHIFT, None, ALU.mult,
                    )
                    nc.vector.tensor_scalar(
                        tposb[0:1, q * P:(q + 1) * P], ps1[:],
                        0.5 * SHIFT, None, ALU.mult,
                    )
                for h in range(2):
                    psb = pst.tile([P, 512], F32, tag="tb", bufs=2)
                    nc.tensor.matmul(
                        psb[:], ones1b[:], tposb[0:1, h * 512:(h + 1) * 512],
                        start=True, stop=True,
                    )
                    nc.scalar.activation(
                        T_bc[:, h * 512:(h + 1) * 512], psb[:], AF.Copy
                    )

            # ================ diffusion (x2) ================
            def diffusion(i, dpool, rhsc_t, den_t, den_src, csrc, cscale,
                          gscale=1.0, do_ag=True):
                """one step: returns dc = (A @ src)_rows-of-core (fp32).

                rhsc_t [P,NQ,D] bf16: src_c/32 (scatter stationary)
                den_t  [P,NS,D] bf16: src/32 (or src; gather stationary)
                den_src: if not None, DRAM ap to load den_t from (after the
                         scatter is issued; sync queue so nothing blocks)
                csrc/cscale: merge-time correction, adds cscale*csrc
                """
                # ---- scatter: St[d, j] = sum_i (src_c/32)[i, d] * M'[i, j]
                # M' comes straight from SBUF (Mnat) — no DMA at all.
                # St stays transposed [d, j]: core block c = cols
                # [c*BC, (c+1)*BC) DMAs straight to bounce rows
                # [c*P, (c+1)*P) — no transposes before the collective.
                S_sbT = None
                with tc.tile_pool(name=f"pscat{i}", bufs=1, space="PSUM") as psc:
                    for half in range(2):
                        S_sbT = dpool.tile([P, 8, 512], BF16, tag="Ssb",
                                           bufs=2)
                        psSt = [
                            psc.tile([P, 512], F32, tag="acc", bufs=8,
                                     name=f"psSt{i}_{half}_{js}")
                            for js in range(8)
                        ]
                        for q in range(NQ):
                            for js in range(8):
                                c0 = half * 4096 + js * 512
                                nc.tensor.matmul(
                                    psSt[js][:], rhsc_t[:, q, :],
                                    Mnat[:, q, c0:c0 + 512],
                                    start=(q == 0), stop=(q == NQ - 1),
                                )
                        for js in range(8):
                            if js % 2 == 0:
                                nc.scalar.activation(
                                    S_sbT[:, js, :], psSt[js][:], AF.Copy
                                )
                            else:
                                nc.vector.tensor_copy(
                                    S_sbT[:, js, :], psSt[js][:]
                                )
                        # this half's 4 core-blocks fly while the other half
                        # is still accumulating
                        # all on the scalar queue: the sync queue must stay
                        # clear so diffusion-2's den load fires the moment
                        # the AllGather lands
                        for cb in range(4):
                            c = half * 4 + cb
                            eng = nc.scalar
                            eng.dma_start(
                                s_bounce[i][c * P:(c + 1) * P, :],
                                S_sbT[:, cb * 2:(cb + 1) * 2, :]
                                .rearrange("p a b -> p (a b)"),
                            )
                nc.gpsimd.collective_compute(
                    "ReduceScatter", ALU.add, replica_groups=RG,
                    ins=[s_bounce[i][:].opt()], outs=[rs_out[i][:].opt()],
                )

                if den_src is not None:
                    nc.sync.dma_start(
                        den_t[:].rearrange("p (c s) d -> p c s d", c=8, s=8),
                        _natp(den_src[:], 8),
                    )

                # ---- gather: Gt[d, i] = sum_j src[j, d] * M'[i, j]
                # pass 0: M'^T j-slices recomputed from sim^T = xbT_js^T @ xcT
                # (fp8, exact for a 0/1 mask), all 64 kept in SBUF; pass 1
                # re-reads them with no DMA at all.
                with tc.tile_pool(name=f"pgat{i}", bufs=1, space="PSUM") as psg:
                    psGt = [
                        psg.tile([P, 512], F32, tag="gacc", bufs=2,
                                 name=f"psGt{i}_{h}")
                        for h in range(2)
                    ]
                    for js in range(NS):
                        if js < NCACHE:
                            MT = mtc[js]
                        elif js < NRES:
                            MT = mtd[js - NCACHE]
                        else:
                            MT = dpool.tile([P, BC], FP8, tag="MTr", bufs=2)
                        if i == 0 or js >= NRES:
                            for h in range(2):
                                rg = psg.tile([P, 512], F32, tag="rg", bufs=6)
                                if js % 2 == 0:
                                    nc.tensor.matmul(
                                        rg[:], xbT[:, js * P:(js + 1) * P],
                                        xcT[:, h * 512:(h + 1) * 512],
                                        start=True, stop=False,
                                    )
                                    nc.tensor.matmul(
                                        rg[:], ones1b[:],
                                        negtb[0:1, h * 512:(h + 1) * 512],
                                        start=False, stop=True,
                                    )
                                    nc.scalar.activation(
                                        MT[:, h * 512:(h + 1) * 512], rg[:],
                                        AF.Sigmoid, scale=1e9,
                                    )
                                else:
                                    nc.tensor.matmul(
                                        rg[:], xbT[:, js * P:(js + 1) * P],
                                        xcT[:, h * 512:(h + 1) * 512],
                                        start=True, stop=True,
                                    )
                                    nc.vector.tensor_tensor(
                                        MT[:, h * 512:(h + 1) * 512], rg[:],
                                        T_bc[:, h * 512:(h + 1) * 512],
                                        ALU.is_ge,
                                    )
                        for h in range(2):
                            nc.tensor.matmul(
                                psGt[h][:], den_t[:, js, :],
                                MT[:, h * 512:(h + 1) * 512],
                                start=(js == 0), stop=(js == NS - 1),
                            )
                    # keep the merge (which waits on the ReduceScatter) from
                    # being scheduled into the js loop's queues — a slow
                    # collective at a queue head stalls every engine
                    tc.no_sync_barrier()
                    GT = dpool.tile([P, 2, 512], F32, tag="GT", bufs=1)
                    for h in range(2):
                        nc.scalar.activation(GT[:, h, :], psGt[h][:], AF.Copy)
                    GTf = GT[:].rearrange("p a b -> p (a b)")

                    # merge rs (already transposed [d, j]) with G^T BEFORE
                    # the transpose pass: one STT + 8 transposes total
                    rsT = dpool.tile([P, BC], BF16, tag="rsT", bufs=1)
                    nc.sync.dma_start(rsT[:], rs_out[i][:])
                    tmpT = dpool.tile([P, BC], BF16, tag="tmpT", bufs=1)
                    nc.vector.scalar_tensor_tensor(
                        tmpT[:], GTf, gscale, rsT[:], ALU.mult, ALU.add,
                    )
                    dc = dpool.tile([P, NQ, D], F32, tag=f"dc{i}")
                    for q in range(NQ):
                        psb = psg.tile([P, P], BF16, tag="rg", bufs=6,
                                       name=f"ptrG{i}_{q}")
                        nc.tensor.transpose(
                            psb[:], tmpT[:, q * P:(q + 1) * P], identb[:]
                        )
                        nc.vector.scalar_tensor_tensor(
                            dc[:, q, :], csrc[:, q, :], cscale, psb[:],
                            ALU.mult, ALU.add,
                        )
                if do_ag:
                    dcb = dpool.tile([P, NQ, D], BF16, tag="dcb", bufs=1)
                    nc.scalar.activation(_r(dcb[:]), _r(dc[:]), AF.Copy)
                    nc.scalar.dma_start(
                        _natp(dcb_dram[:], 1),
                        dcb[:].rearrange("p (c s) d -> p c s d", c=1, s=8),
                    )
                    nc.gpsimd.collective_compute(
                        "AllGather", ALU.bypass, replica_groups=RG,
                        ins=[dcb_dram[:].opt()], outs=[den_ag[:].opt()],
                    )
                return dc

            with tc.tile_pool(name="dif", bufs=1) as dpool:
                # middle of the transposed mask (slices NCACHE..NRES-1),
                # allocated here so it reuses the sim pools' freed SBUF
                mtd = [
                    dpool.tile([P, BC], FP8, name=f"mtd{js}")
                    for js in range(NRES - NCACHE)
                ]
                dc1 = diffusion(0, dpool, fbc32, fb32, None, fbc32, -2.0)
                # operands for pass 2 (diff1 arrives bf16 via AllGather);
                # den2 stays unscaled, the gather merge divides by 32.
                rhsc2 = dpool.tile([P, NQ, D], BF16, tag="rhsc2")
                nc.scalar.activation(_r(rhsc2[:]), _r(dc1[:]), AF.Copy, scale=1 / 32)
                den2 = dpool.tile([P, NS, D], BF16, tag="den2")

                dc2 = diffusion(1, dpool, rhsc2, den2, den_ag, dc1, -1.0 / 16.0,
                                gscale=1.0 / 32.0, do_ag=False)

                # ---- phase 7: z_c = x_c + 0.1 * geo_c/||geo_c||; AllGather
                # z^T in two column-halves so phase 8 can start scoring the
                # first half while the second is still in flight.  Fully
                # per-q pipelined: no whole-tile sqrt/recip barrier.
                n2g = dpool.tile([P, NQ], F32)
                ng = dpool.tile([P, NQ], F32)
                rg01 = dpool.tile([P, NQ], F32)
                zbc = dpool.tile([P, NQ, D], BF16)
                zcT = dpool.tile([P, BC], BF16, tag="zcT")
                with tc.tile_pool(name="psz", bufs=1, space="PSUM") as psz:
                    for q in range(NQ):
                        sq = dpool.tile([P, D], F32, tag="sqg", bufs=2)
                        nc.vector.scalar_tensor_tensor(
                            sq[:], dc2[:, q, :], 1.0, dc2[:, q, :],
                            ALU.mult, ALU.mult, accum_out=n2g[:, q:q + 1],
                        )
                        nc.scalar.activation(
                            ng[:, q:q + 1], n2g[:, q:q + 1], AF.Sqrt
                        )
                        rgn = dpool.tile([P, 1], F32, tag="rgn", bufs=2)
                        nc.vector.reciprocal(rgn[:], ng[:, q:q + 1])
                        nc.vector.tensor_scalar(
                            rg01[:, q:q + 1], rgn[:], 0.1, None, ALU.mult
                        )
                        nc.vector.scalar_tensor_tensor(
                            zbc[:, q, :], dc2[:, q, :], rg01[:, q:q + 1],
                            xc_nat[:, q, :], ALU.mult, ALU.add,
                        )
                        psq = psz.tile([P, P], BF16, tag="ptr", bufs=2)
                        nc.tensor.transpose(psq[:], zbc[:, q, :], identb[:])
                        if q % 2 == 0:
                            nc.scalar.activation(
                                zcT[:, q * P:(q + 1) * P], psq[:], AF.Copy
                            )
                        else:
                            nc.vector.tensor_copy(
                                zcT[:, q * P:(q + 1) * P], psq[:]
                            )
                nc.scalar.dma_start(zct_dram[:], zcT[:])
                nc.gpsimd.collective_compute(
                    "AllGather", ALU.bypass, replica_groups=RG,
                    ins=[zct_dram[:].opt()], outs=[zt_ag[:].opt()],
                )

            # ================ phase 8: softmax attention ================
            with (
                tc.tile_pool(name="p8", bufs=1) as p8,
                tc.tile_pool(name="ps8", bufs=1, space="PSUM") as psp,
            ):
                zT = p8.tile([P, B], BF16)
                for c in range(NCORES):
                    eng = nc.sync if c % 2 == 0 else nc.scalar
                    eng.dma_start(
                        zT[:, c * BC:(c + 1) * BC],
                        zt_ag[c * P:(c + 1) * P, :],
                    )
                jc_list = list(range(NS))
                # keep the PE HAM window busy while the z AllGather lands so
                # the score matmuls start at 2.4 GHz, not 1.2.  Real matmuls,
                # not transposes — transpose-mode does not count as PE-busy
                # for the HAM activity window.
                wsc = p8.tile([P, P], F32, tag="warm")
                for w in range(48):
                    psw = psp.tile([P, P], F32, tag="psw", bufs=1)
                    nc.tensor.matmul(
                        psw[:], identb[:], identb[:], start=True, stop=True,
                    )
                    if w == 47:
                        nc.vector.tensor_copy(wsc[:], psw[:])
                for grp in range(2):
                    psOUT = psp.tile([P, 512], F32, tag="psOUT", bufs=1)
                    psS1 = psp.tile([1, 512], F32, tag="psS1", bufs=1)
                    sacc = p8.tile([P, 512], F32, tag="sacc", bufs=2)
                    nc.vector.memset(sacc[:], 0.0)
                    # software-pipelined: scores(jp) | exp(jp-1) | V+denom
                    # accumulate(jp-2).  Denominator split: slice 0 on PE
                    # (ones matmul), slice 1 on DVE (sacc accumulate).
                    psTt, Ptt = {}, {}
                    for step in range(NS // 2 + 2):
                        if step < NS // 2:
                            jp = step
                            psTt[jp] = psp.tile([P, 2, 512], F32, tag="psT",
                                                bufs=2, name=f"psT{grp}_{jp}")
                            for u in range(2):
                                jc = jc_list[jp * 2 + u]
                                nc.tensor.matmul(
                                    psTt[jp][:, u, :],
                                    zT[:, jc * P:(jc + 1) * P],
                                    xcT[:, grp * 512:(grp + 1) * 512],
                                    start=True, stop=True,
                                )
                        if 1 <= step <= NS // 2:
                            jp = step - 1
                            Ptt[jp] = p8.tile([P, 2, 512], BF16, tag="Pt",
                                              bufs=4, name=f"Pt{grp}_{jp}")
                            psT = psTt.pop(jp)
                            nc.scalar.activation(
                                Ptt[jp][:].rearrange("p a b -> p (a b)"),
                                psT[:].rearrange("p a b -> p (a b)"),
                                AF.Exp, scale=10.0,
                            )
                        if step >= 2:
                            jp = step - 2
                            Pt = Ptt.pop(jp)
                            for u in range(2):
                                pos = jp * 2 + u
                                jc = jc_list[pos]
                                nc.tensor.matmul(
                                    psOUT[:], fb32[:, jc, :], Pt[:, u, :],
                                    start=(pos == 0), stop=(pos == NS - 1),
                                )
                            nc.tensor.matmul(
                                psS1[:], ones_col[:], Pt[:, 0, :],
                                start=(jp == 0), stop=(jp == NS // 2 - 1),
                            )
                            nc.vector.scalar_tensor_tensor(
                                sacc[:], Pt[:, 1, :], 1.0, sacc[:],
                                ALU.mult, ALU.add,
                            )
                    # fold the PE partial into sacc row 0, then reduce over
                    # partitions via 4 PE transposes
                    nc.vector.tensor_tensor(
                        sacc[0:1, :], sacc[0:1, :], psS1[:], ALU.add,
                    )
                    # denom: reduce sacc over partitions via 4 PE transposes
                    s1nat = p8.tile([P, 4], F32, tag="s1nat", bufs=2)
                    scr = p8.tile([P, P], F32, tag="scr", bufs=2)
                    for b in range(4):
                        psB = psp.tile([P, P], F32, tag="psB", bufs=1)
                        nc.tensor.transpose(
                            psB[:], sacc[:, b * P:(b + 1) * P], identf[:]
                        )
                        nc.vector.tensor_scalar(
                            scr[:], psB[:], 1.0, 0.0, ALU.mult, ALU.add,
                            accum_out=s1nat[:, b:b + 1],
                        )
                    rnat = p8.tile([P, 4], F32, tag="rnat", bufs=2)
                    nc.vector.reciprocal(rnat[:], s1nat[:])
                    rnat32 = p8.tile([P, 4], F32, tag="rnat32", bufs=2)
                    nc.vector.tensor_scalar(
                        rnat32[:], rnat[:], 32.0, None, ALU.mult
                    )
                    OUT_sb = p8.tile([P, 512], F32, tag="OUTsb", bufs=2)
                    nc.scalar.activation(OUT_sb[:], psOUT[:], AF.Copy)
                    for b in range(4):
                        psB = psp.tile([P, P], F32, tag="psB", bufs=1)
                        nc.tensor.transpose(
                            psB[:], OUT_sb[:, b * P:(b + 1) * P], identf[:]
                        )
                        ob = p8.tile([P, D], F32, tag="ob", bufs=2)
                        nc.vector.tensor_scalar(
                            ob[:], psB[:], rnat32[:, b:b + 1], None, ALU.mult
                        )
                        nc.scalar.dma_start(
                            out_ext[grp * 512 + b * P: grp * 512 + (b + 1) * P, :],
                            ob[:],
                        )

    nc.finalize()
    return nc


_NC_CACHE = None


def kernel(features: np.ndarray) -> np.ndarray:
    global _NC_CACHE
    features = np.ascontiguousarray(np.asarray(features, np.float32))
    assert features.shape == (B, D), features.shape
    if _NC_CACHE is None:
        _NC_CACHE = build()
    # per-1024-block p-major relabeling: device row c*1024 + p*8 + s holds
    # node c*1024 + s*128 + p, so every [N*128, D] DMA runs with 4 KB
    # contiguous descriptors.  The computation is permutation-equivariant;
    # device output rows come back in natural node order (out writes are
    # natural-layout), so no un-permute is needed.
    feat_dev = np.ascontiguousarray(
        features.reshape(NCORES, 8, P, D).swapaxes(1, 2).reshape(B, D)
    )
    in_maps = [
        {
            "feat": feat_dev,
            "featc": feat_dev[c * BC:(c + 1) * BC].copy(),
        }
        for c in range(NCORES)
    ]
    res = run_bass_kernel_spmd(_NC_CACHE, in_maps, core_ids=list(range(NCORES)))
    return np.concatenate(
        [np.asarray(res.results[c]["out"], np.float32) for c in range(NCORES)],
        axis=0,
    )

